# revision 3
# baseline (speedup 1.0000x reference)
"""Mixtral-style GQA attention block, tensor-parallel over 8 NeuronCores.

Sharding: core i owns q heads 4i..4i+3 and kv head i (GQA group == 4, so the
kv head's whole group lives on one core).  w_qkv is column-sharded by head,
w_o is row-sharded; the only collective is an AllGather of the per-core
attention outputs (bf16, 2MB per core).  Each core then computes a disjoint
512-column slice of the final output, so the host-side unshard is a pure
concatenation.

v2 layout: the whole rep is software-pipelined in emission order —
QKV runs in t-quarters with attention tq-blocks, the two AllGathers and
o_proj tiles interleaved between them so softmax exp (Act engine) and the
collectives hide under GEMM streaming.  PV streams V||ones against a
stationary probability tile, so softmax row-sums come free in psum column
128 (no separate ones-matmul, no broadcast matmul); normalization is a
per-partition scalar multiply and the [tq, d] result is flipped back to
[d, tq] with xbar DMA transposes.

All matmuls run in bf16 (fp32 PSUM accumulation); softmax runs in fp32
without max-subtraction (scores are ~N(0,1) by construction, exp cannot
overflow).
"""

import numpy as np
import ml_dtypes
from contextlib import ExitStack

import concourse.bass as bass
import concourse.mybir as mybir
import concourse.tile as tile
from concourse import bacc
from concourse.bass_utils import run_bass_kernel_spmd

P = 128
HID = 4096
D = 128
QH = 4                      # local q heads per core
NB = 6                      # projection M-blocks: q0..q3, k, v
KC = HID // P               # contraction chunks over hidden dim
N_CORES = 8
SCALE = float(D) ** -0.5
NEG = -1.0e30

dt = mybir.dt
bf16 = ml_dtypes.bfloat16

F32 = dt.float32
BF16 = dt.bfloat16


def build_nc(t_len=2048, phases=3, reps=1):
    TCH = t_len // P            # token chunks (16)
    TE = t_len // 8             # eighth width (256) — qkv chunk granularity
    TQ = t_len // 4             # attention block width (512)
    TH = t_len // 2             # half width (1024) — allgather granularity
    QT = 256                    # o_proj t-tile width
    NJ = TQ // P                # tq chunks per attention block (4)
    WQ = NB * P                 # 768
    WO = QH * P                 # 512
    VW = 2 * P                  # v chunk stride: xbar-transpose dests need 256B align
    SW = P + 8                  # streamed width per v chunk: v(128) + ones + 7 pad

    nc = bacc.Bacc("TRN2", target_bir_lowering=False, debug=False,
                   num_devices=N_CORES)

    hiddenT = nc.dram_tensor("hiddenT", [HID, t_len], BF16, kind="ExternalInput").ap()
    wqkvT = nc.dram_tensor("wqkvT", [HID, WQ], BF16, kind="ExternalInput").ap()
    woT = nc.dram_tensor("woT", [HID, WO], BF16, kind="ExternalInput").ap()
    cos2 = nc.dram_tensor("cos2", [P, t_len], F32, kind="ExternalInput").ap()
    sin2 = nc.dram_tensor("sin2", [P, t_len], F32, kind="ExternalInput").ap()
    maskd = nc.dram_tensor("maskd", [P, P], F32, kind="ExternalInput").ap()
    outp = nc.dram_tensor("outp", [WO, t_len], F32, kind="ExternalOutput").ap()

    with tile.TileContext(nc) as tc:
        with ExitStack() as whole:
            persist = whole.enter_context(tc.tile_pool(name="persist", bufs=1))
            dram = whole.enter_context(tc.tile_pool(name="dram", bufs=1, space="DRAM"))

            # ---- constants ----
            cos2_sb = persist.tile([P, t_len], F32, tag="cos2")
            sin2_sb = persist.tile([P, t_len], F32, tag="sin2")
            mask_sb = persist.tile([P, P], F32, tag="mask")
            nc.sync.dma_start(cos2_sb[:], cos2[:])
            nc.sync.dma_start(sin2_sb[:], sin2[:])
            nc.sync.dma_start(mask_sb[:], maskd[:])

            # ---- persistent activations ----
            qk_sb = [persist.tile([P, t_len], BF16, tag=f"qk{mb}", name=f"qk{mb}")
                     for mb in range(5)]
            # v chunks interleaved with a ones column: chunk c at
            # [:, c*VW : c*VW+P], ones at [:, c*VW+P]
            v_sb = persist.tile([P, TCH * VW], BF16, tag="v", name="v_sb")
            for gc in range(TCH):
                nc.vector.memset(v_sb[:, gc * VW + P:gc * VW + P + 1], 1.0)
                nc.vector.memset(v_sb[:, gc * VW + P + 1:gc * VW + SW], 0.0)

            # ---- persistent weight caches (shared by all reps) ----
            JC = N_CORES * QH
            wqkv_sb = persist.tile([P, KC * WQ], BF16, tag="wqkv")
            wqkv_src = wqkvT.rearrange("(c p) w -> p c w", p=P)
            wqkv_dst = wqkv_sb.rearrange("p (c w) -> p c w", w=WQ)
            for c0 in range(0, KC, 8):
                nc.scalar.dma_start(wqkv_dst[:, c0:c0 + 8, :],
                                    wqkv_src[:, c0:c0 + 8, :])
            wo_sb = persist.tile([P, JC * WO], BF16, tag="wo")
            wo_src = woT.rearrange("(c p) w -> p c w", p=P)
            wo_dst = wo_sb.rearrange("p (c w) -> p c w", w=WO)
            for c0 in range(0, JC, 8):
                nc.scalar.dma_start(wo_dst[:, c0:c0 + 8, :],
                                    wo_src[:, c0:c0 + 8, :])

            for rep in range(reps):
              with ExitStack() as rep_stack:
                attn_bounce = [dram.tile([QH * P, TH], BF16,
                                         tag=f"attn_bounce{rep}_{hb}",
                                         name=f"attn_bounce{rep}_{hb}")
                               for hb in range(2)]
                gathered = [dram.tile([N_CORES * QH * P, TH], BF16,
                                      tag=f"gathered{rep}_{hb}",
                                      name=f"gathered{rep}_{hb}",
                                      addr_space="Shared")
                            for hb in range(2)]

                # ---- attention pools (live for the whole rep) ----
                st_pool = rep_stack.enter_context(
                    tc.tile_pool(name="st_psum", bufs=2, space="PSUM"))
                pv_pool = rep_stack.enter_context(
                    tc.tile_pool(name="pv_psum", bufs=2, space="PSUM"))
                p_pool = rep_stack.enter_context(tc.tile_pool(name="pstrips", bufs=20))
                misc = rep_stack.enter_context(tc.tile_pool(name="attn_misc", bufs=4))
                ts_pool = rep_stack.enter_context(tc.tile_pool(name="tstage", bufs=2))

                # ---- phase-1 pools (closed after the last qkv eighth) ----
                ph1 = ExitStack()
                hid_pool = ph1.enter_context(tc.tile_pool(name="hid", bufs=2))
                ps_pool = ph1.enter_context(
                    tc.tile_pool(name="proj_psum", bufs=4, space="PSUM"))
                stage = ph1.enter_context(tc.tile_pool(name="stage", bufs=5))

                hid_src = hiddenT.rearrange("(c p) t -> p c t", p=P)
                hid_tiles = {}

                def load_hid(e):
                    if e >= 8 or e in hid_tiles:
                        return
                    ta, tb = e * TE, (e + 1) * TE
                    hid_e = hid_pool.tile([P, KC * TE], BF16, tag="hidc",
                                          name=f"hid{rep}_{e}")
                    hid_dst = hid_e.rearrange("p (c t) -> p c t", t=TE)
                    for c0 in range(0, KC, 8):
                        nc.sync.dma_start(hid_dst[:, c0:c0 + 8, :],
                                          hid_src[:, c0:c0 + 8, ta:tb])
                    hid_tiles[e] = hid_e

                def emit_qkv_eighth(e):
                    ta, tb = e * TE, (e + 1) * TE
                    load_hid(e)
                    hid_e = hid_tiles.pop(e)
                    load_hid(e + 1)          # prefetch next eighth
                    for pair, mbs in enumerate([(4, 5), (0, 1), (2, 3)]):
                        psums = []
                        for u in range(2):
                            pt = ps_pool.tile([P, TE], F32, tag="pj",
                                              name=f"pj{rep}_{e}_{pair}_{u}")
                            psums.append(pt)
                        for c in range(KC):
                            for u in range(2):
                                mb = mbs[u]
                                lhsT = wqkv_sb[:, c * WQ + mb * P:c * WQ + (mb + 1) * P]
                                nc.tensor.matmul(
                                    psums[u][:, :], lhsT=lhsT,
                                    rhs=hid_e[:, c * TE:(c + 1) * TE],
                                    start=(c == 0), stop=(c == KC - 1))
                        for u in range(2):
                            mb = mbs[u]
                            if mb < 5:
                                # rope: qk[d] = raw[d]*cos2[d] + raw[(d+64)%128]*sin2[d]
                                raw = stage.tile([P, TE], F32, tag="stg", name="raw")
                                nc.vector.tensor_copy(raw[:], psums[u][:])
                                rot = stage.tile([P, TE], F32, tag="stg", name="rot")
                                nc.gpsimd.dma_start(rot[0:64, :], raw[64:128, :])
                                nc.gpsimd.dma_start(rot[64:128, :], raw[0:64, :])
                                t1 = stage.tile([P, TE], F32, tag="stg", name="t1")
                                nc.vector.tensor_mul(t1[:], raw[:], cos2_sb[:, ta:tb])
                                t2 = stage.tile([P, TE], F32, tag="stg", name="t2")
                                nc.vector.tensor_mul(t2[:], rot[:], sin2_sb[:, ta:tb])
                                nc.vector.tensor_add(qk_sb[mb][:, ta:tb], t1[:], t2[:])
                            else:
                                # v: evacuate bf16 [d, t], transpose each [d, tk]
                                # chunk into v_sb's [tk, d] slot (ones col kept)
                                vstg = stage.tile([P, TE], BF16, tag="vstg",
                                                  name="vstg")
                                nc.vector.tensor_copy(vstg[:], psums[u][:])
                                for ct in range(TE // P):
                                    gc = e * (TE // P) + ct
                                    nc.sync.dma_start_transpose(
                                        v_sb[:, gc * VW:gc * VW + P],
                                        vstg[:, ct * P:(ct + 1) * P])

                kT = qk_sb[4]

                def emit_attn_head(b, h):
                    """Attention for tq block b (tq0=512b), local q head h."""
                    tq0 = b * TQ
                    hb = b // 2
                    qT = qk_sb[h]
                    ncv = (tq0 + TQ) // P     # contributing tk chunks
                    # ---- pass A: scores + exp -> P strips ----
                    strips = []
                    for c in range(ncv):
                        off = max(tq0, P * c)
                        w = tq0 + TQ - off
                        st = st_pool.tile([P, TQ], F32, tag="st",
                                          name=f"st{rep}_{b}_{h}_{c}")
                        nc.tensor.matmul(st[:, 0:w],
                                         lhsT=kT[:, c * P:(c + 1) * P],
                                         rhs=qT[:, off:off + w],
                                         start=True, stop=True)
                        if P * c >= tq0:
                            # strip starts on the diagonal: mask tq < tk
                            nc.vector.tensor_add(st[:, 0:P], st[:, 0:P], mask_sb[:])
                        pt = p_pool.tile([P, TQ], BF16, tag="p",
                                         name=f"p{rep}_{b}_{h}_{c}")
                        nc.scalar.activation(pt[:, 0:w], st[:, 0:w],
                                             mybir.ActivationFunctionType.Exp,
                                             scale=SCALE)
                        if phases == 4 and h == 0 and b == 0 and c < 2:
                            dp = misc.tile([P, TQ], F32, tag="dbgp", name="dp")
                            nc.vector.tensor_copy(dp[:, 0:w], pt[:, 0:w])
                            nc.scalar.dma_start(
                                outp[(2 + c) * P:(3 + c) * P, 0:w], dp[:, 0:w])
                        strips.append((pt, off, w))
                    # ---- pass B: stationary-P PV, row sums free in col 128 ----
                    tstg = ts_pool.tile([P, TQ], BF16, tag="ts", name="tstg")
                    for j in range(NJ):
                        jg = tq0 // P + j     # global tq chunk
                        pv = pv_pool.tile([P, 512], F32, tag="pv",
                                          name=f"pv{rep}_{b}_{h}_{j}")
                        for c in range(jg + 1):
                            pt, off, w = strips[c]
                            a = jg * P - off
                            nc.tensor.matmul(
                                pv[:, 0:SW],
                                lhsT=pt[:, a:a + P],
                                rhs=v_sb[:, c * VW:c * VW + SW],
                                start=(c == 0), stop=(c == jg))
                        # normalize rows by col-P sums; result is [tq, d]
                        inv = misc.tile([P, 1], F32, tag="inv", name="inv")
                        nc.vector.reciprocal(inv[:], pv[:, P:P + 1])
                        normed = misc.tile([P, P], BF16, tag="nrm", name="normed")
                        nc.vector.tensor_scalar_mul(normed[:], pv[:, 0:P], inv[:])
                        if phases == 4 and h == 0:
                            # debug: normed chunk (no transpose) + l column
                            dn = misc.tile([P, P], F32, tag="dbgn", name="dn")
                            nc.vector.tensor_copy(dn[:], normed[:])
                            nc.scalar.dma_start(outp[0:P, jg * P:(jg + 1) * P],
                                                dn[:])
                            dl = misc.tile([P, 1], F32, tag="dbgl", name="dl")
                            nc.vector.tensor_copy(dl[:], pv[:, P:P + 1])
                            nc.scalar.dma_start(outp[P:2 * P, jg * P:jg * P + 1],
                                                dl[:])
                        # flip back to [d, tq]
                        nc.sync.dma_start_transpose(tstg[:, j * P:(j + 1) * P],
                                                    normed[:])
                    nc.scalar.dma_start(
                        attn_bounce[hb][h * P:(h + 1) * P,
                                        tq0 - hb * TH:tq0 - hb * TH + TQ],
                        tstg[:])
                    if phases == 2:
                        # debug: dump per-head attention output to outp (as f32)
                        dbg = misc.tile([P, TQ], F32, tag="dbgf", name="dbgf")
                        nc.vector.tensor_copy(dbg[:], tstg[:])
                        nc.scalar.dma_start(outp[h * P:(h + 1) * P, tq0:tq0 + TQ],
                                            dbg[:])

                def emit_gather(hb):
                    nc.gpsimd.collective_compute(
                        "AllGather",
                        mybir.AluOpType.bypass,
                        ins=[attn_bounce[hb][:]],
                        outs=[gathered[hb][:]],
                        replica_groups=[list(range(N_CORES))],
                    )

                # ---- phase-3 state (pools entered after ph1 closes) ----
                p3 = {}

                def open_phase3_pools():
                    ag_pool = rep_stack.enter_context(tc.tile_pool(name="ag", bufs=4))
                    po_pool = rep_stack.enter_context(
                        tc.tile_pool(name="oproj_psum", bufs=4, space="PSUM"))
                    ostg = rep_stack.enter_context(tc.tile_pool(name="ostg", bufs=3))
                    p3.update(ag_pool=ag_pool, po_pool=po_pool, ostg=ostg)

                def emit_oproj_tile(tq):
                    ghalf = (tq * QT) // TH
                    qa = tq * QT - ghalf * TH      # offset within gathered
                    ag_pool, po_pool, ostg = p3["ag_pool"], p3["po_pool"], p3["ostg"]
                    psums = []
                    for mb in range(QH):
                        pt = po_pool.tile([P, QT], F32, tag="po",
                                          name=f"po{rep}_{tq}_{mb}")
                        psums.append(pt)
                    ag_src = gathered[ghalf].rearrange("(c p) t -> p c t", p=P)
                    for cg in range(0, JC, 4):
                        ag_t = ag_pool.tile([P, 4 * QT], BF16, tag="ag",
                                            name="ag_t")
                        ag_dst = ag_t.rearrange("p (c t) -> p c t", t=QT)
                        eng = nc.scalar if (cg // 4) % 2 else nc.sync
                        eng.dma_start(ag_dst[:, :, :],
                                      ag_src[:, cg:cg + 4, qa:qa + QT])
                        for ci in range(4):
                            c = cg + ci
                            for mb in range(QH):
                                lhsT = wo_sb[:, c * WO + mb * P:c * WO + (mb + 1) * P]
                                nc.tensor.matmul(
                                    psums[mb][:, :], lhsT=lhsT,
                                    rhs=ag_t[:, ci * QT:(ci + 1) * QT],
                                    start=(c == 0), stop=(c == JC - 1))
                    for mb in range(QH):
                        ob = ostg.tile([P, QT], F32, tag="ob", name="ob")
                        nc.vector.tensor_copy(ob[:], psums[mb][:])
                        nc.scalar.dma_start(
                            outp[mb * P:(mb + 1) * P, tq * QT:(tq + 1) * QT],
                            ob[:])

                # ================= pipelined emission =================
                for e in range(4):
                    emit_qkv_eighth(e)
                full = phases == 3
                if phases >= 2:
                    for h in range(QH):
                        emit_attn_head(0, h)
                emit_qkv_eighth(4)
                emit_qkv_eighth(5)
                if phases >= 2:
                    for h in range(QH):
                        emit_attn_head(1, h)
                    if full:
                        emit_gather(0)
                emit_qkv_eighth(6)
                emit_qkv_eighth(7)
                ph1.close()
                if phases == 1:
                    # debug: dump rope'd q heads + k to outp (rows 0..511 = q0..q3)
                    dbg_pool = rep_stack.enter_context(
                        tc.tile_pool(name="dbg", bufs=1))
                    for h in range(4):
                        dtile = dbg_pool.tile([P, t_len], F32, tag=f"dbg{h}")
                        nc.vector.tensor_copy(dtile[:], qk_sb[h][:])
                        nc.scalar.dma_start(outp[h * P:(h + 1) * P, :], dtile[:])
                if phases >= 2:
                    if full:
                        open_phase3_pools()
                    for h in range(QH):
                        emit_attn_head(2, h)
                    emit_attn_head(3, 0)
                    emit_attn_head(3, 1)
                    if full:
                        emit_oproj_tile(0)
                    emit_attn_head(3, 2)
                    emit_attn_head(3, 3)
                    if full:
                        emit_gather(1)
                        for tq in range(1, t_len // QT):
                            emit_oproj_tile(tq)

    nc.compile()
    return nc


def make_inputs(positions, hidden_states, w_qkv, w_o):
    """Host-side shard + relayout.  Returns per-core input maps."""
    half = D // 2
    inv_freq = 1.0 / (1e6 ** (np.arange(0, half, dtype=np.float32) / half))
    freqs = positions.astype(np.float32)[:, None] * inv_freq[None, :]
    cosT = np.cos(freqs).T.astype(np.float32)      # [64, T]
    sinT = np.sin(freqs).T.astype(np.float32)
    cos2 = np.ascontiguousarray(np.concatenate([cosT, cosT], axis=0))
    sin2 = np.ascontiguousarray(np.concatenate([-sinT, sinT], axis=0))

    ii = np.arange(P)
    maskd = np.where(ii[None, :] >= ii[:, None], 0.0, NEG).astype(np.float32)

    hiddenT = np.ascontiguousarray(hidden_states.T).astype(bf16)

    q_size = 32 * D
    in_maps = []
    for i in range(N_CORES):
        rows = np.concatenate([
            w_qkv[QH * P * i:QH * P * (i + 1)],                      # 4 q heads
            w_qkv[q_size + P * i:q_size + P * (i + 1)],              # k head
            w_qkv[q_size + 8 * D + P * i:q_size + 8 * D + P * (i + 1)],  # v head
        ], axis=0)
        wqkvT_i = np.ascontiguousarray(rows.T).astype(bf16)
        woT_i = np.ascontiguousarray(w_o[QH * P * i:QH * P * (i + 1), :].T).astype(bf16)
        in_maps.append({
            "hiddenT": hiddenT,
            "wqkvT": wqkvT_i,
            "woT": woT_i,
            "cos2": cos2,
            "sin2": sin2,
            "maskd": maskd,
        })
    return in_maps


def assemble(results, t_len=2048):
    final = np.empty((t_len, N_CORES * QH * P), dtype=np.float32)
    for i in range(N_CORES):
        final[:, QH * P * i:QH * P * (i + 1)] = results[i]["outp"].T
    return final


def kernel(positions, hidden_states, w_qkv, w_o):
    positions = np.asarray(positions)
    hidden_states = np.asarray(hidden_states, dtype=np.float32)
    w_qkv = np.asarray(w_qkv, dtype=np.float32)
    w_o = np.asarray(w_o, dtype=np.float32)
    t_len = hidden_states.shape[0]

    nc = build_nc(t_len)
    in_maps = make_inputs(positions, hidden_states, w_qkv, w_o)
    res = run_bass_kernel_spmd(nc, in_maps, list(range(N_CORES)))
    return assemble(res.results, t_len)


# revision 4
# speedup vs baseline: 1.3825x; 1.3825x over previous
"""Mixtral-style GQA attention block, tensor-parallel over 8 NeuronCores.

Sharding: core i owns q heads 4i..4i+3 and kv head i (GQA group == 4, so the
kv head's whole group lives on one core).  w_qkv is column-sharded by head,
w_o is row-sharded; the only collective is an AllGather of the per-core
attention outputs (bf16, 2MB per core).  Each core then computes a disjoint
512-column slice of the final output, so the host-side unshard is a pure
concatenation.

v2 layout: the whole rep is software-pipelined in emission order —
QKV runs in t-quarters with attention tq-blocks, the two AllGathers and
o_proj tiles interleaved between them so softmax exp (Act engine) and the
collectives hide under GEMM streaming.  PV streams V||ones against a
stationary probability tile, so softmax row-sums come free in psum column
128 (no separate ones-matmul, no broadcast matmul); normalization is a
per-partition scalar multiply and the [tq, d] result is flipped back to
[d, tq] with xbar DMA transposes.

All matmuls run in bf16 (fp32 PSUM accumulation); softmax runs in fp32
without max-subtraction (scores are ~N(0,1) by construction, exp cannot
overflow).
"""

import numpy as np
import ml_dtypes
from contextlib import ExitStack

import concourse.bass as bass
import concourse.mybir as mybir
import concourse.tile as tile
from concourse import bacc
from concourse.bass_utils import run_bass_kernel_spmd

P = 128
HID = 4096
D = 128
QH = 4                      # local q heads per core
NB = 6                      # projection M-blocks: q0..q3, k, v
KC = HID // P               # contraction chunks over hidden dim
N_CORES = 8
SCALE = float(D) ** -0.5
NEG = -1.0e30

dt = mybir.dt
bf16 = ml_dtypes.bfloat16

F32 = dt.float32
BF16 = dt.bfloat16


def build_nc(t_len=2048, phases=3, reps=1):
    TCH = t_len // P            # token chunks (16)
    TE = t_len // 8             # eighth width (256) — qkv chunk granularity
    TQ = t_len // 4             # attention block width (512)
    TH = t_len // 2             # half width (1024) — allgather granularity
    QT = 256                    # o_proj t-tile width
    NJ = TQ // P                # tq chunks per attention block (4)
    WQ = NB * P                 # 768
    WO = QH * P                 # 512
    VW = 2 * P                  # v chunk stride: xbar-transpose dests need 256B align
    SW = P + 8                  # streamed width per v chunk: v(128) + ones + 7 pad

    nc = bacc.Bacc("TRN2", target_bir_lowering=False, debug=False,
                   num_devices=N_CORES)

    hiddenT = nc.dram_tensor("hiddenT", [HID, t_len], BF16, kind="ExternalInput").ap()
    wqkvT = nc.dram_tensor("wqkvT", [HID, WQ], BF16, kind="ExternalInput").ap()
    woT = nc.dram_tensor("woT", [HID, WO], BF16, kind="ExternalInput").ap()
    cos2 = nc.dram_tensor("cos2", [P, t_len], F32, kind="ExternalInput").ap()
    sin2 = nc.dram_tensor("sin2", [P, t_len], F32, kind="ExternalInput").ap()
    maskd = nc.dram_tensor("maskd", [P, P], F32, kind="ExternalInput").ap()
    outp = nc.dram_tensor("outp", [WO, t_len], F32, kind="ExternalOutput").ap()

    with tile.TileContext(nc) as tc:
        with ExitStack() as whole:
            persist = whole.enter_context(tc.tile_pool(name="persist", bufs=1))
            dram = whole.enter_context(tc.tile_pool(name="dram", bufs=1, space="DRAM"))

            # ---- constants ----
            cos2_sb = persist.tile([P, t_len], F32, tag="cos2")
            sin2_sb = persist.tile([P, t_len], F32, tag="sin2")
            mask_sb = persist.tile([P, P], F32, tag="mask")
            nc.sync.dma_start(cos2_sb[:], cos2[:])
            nc.sync.dma_start(sin2_sb[:], sin2[:])
            nc.sync.dma_start(mask_sb[:], maskd[:])

            # ---- persistent activations ----
            qk_sb = [persist.tile([P, t_len], BF16, tag=f"qk{mb}", name=f"qk{mb}")
                     for mb in range(5)]
            # v chunks interleaved with a ones column: chunk c at
            # [:, c*VW : c*VW+P], ones at [:, c*VW+P]
            v_sb = persist.tile([P, TCH * VW], BF16, tag="v", name="v_sb")
            for gc in range(TCH):
                nc.vector.memset(v_sb[:, gc * VW + P:gc * VW + P + 1], 1.0)
                nc.vector.memset(v_sb[:, gc * VW + P + 1:gc * VW + SW], 0.0)

            # ---- persistent weight caches (shared by all reps) ----
            JC = N_CORES * QH
            wqkv_sb = persist.tile([P, KC * WQ], BF16, tag="wqkv")
            wqkv_src = wqkvT.rearrange("(c p) w -> p c w", p=P)
            wqkv_dst = wqkv_sb.rearrange("p (c w) -> p c w", w=WQ)
            for c0 in range(0, KC, 8):
                nc.scalar.dma_start(wqkv_dst[:, c0:c0 + 8, :],
                                    wqkv_src[:, c0:c0 + 8, :])
            wo_sb = persist.tile([P, JC * WO], BF16, tag="wo")
            wo_src = woT.rearrange("(c p) w -> p c w", p=P)
            wo_dst = wo_sb.rearrange("p (c w) -> p c w", w=WO)
            for c0 in range(0, JC, 8):
                nc.scalar.dma_start(wo_dst[:, c0:c0 + 8, :],
                                    wo_src[:, c0:c0 + 8, :])

            for rep in range(reps):
              with ExitStack() as rep_stack:
                attn_bounce = [dram.tile([QH * P, TH], BF16,
                                         tag=f"attn_bounce{rep}_{hb}",
                                         name=f"attn_bounce{rep}_{hb}")
                               for hb in range(2)]
                gathered = [dram.tile([N_CORES * QH * P, TH], BF16,
                                      tag=f"gathered{rep}_{hb}",
                                      name=f"gathered{rep}_{hb}",
                                      addr_space="Shared")
                            for hb in range(2)]

                # ---- all pools allocated at rep start in fixed order, so
                # each rep's pools land on the same regions and WAR only
                # against their own previous-rep usage (QKV psum frees early,
                # o_proj psum frees last — no cross-pool collisions)
                gemm_pool = rep_stack.enter_context(
                    tc.tile_pool(name="gemm_psum", bufs=4, space="PSUM"))
                st_pool = rep_stack.enter_context(
                    tc.tile_pool(name="st_psum", bufs=2, space="PSUM"))
                pv_pool = rep_stack.enter_context(
                    tc.tile_pool(name="pv_psum", bufs=2, space="PSUM"))
                p_pool = rep_stack.enter_context(tc.tile_pool(name="pstrips", bufs=20))
                misc = rep_stack.enter_context(tc.tile_pool(name="attn_misc", bufs=4))
                ts_pool = rep_stack.enter_context(tc.tile_pool(name="tstage", bufs=2))
                hid_pool = rep_stack.enter_context(tc.tile_pool(name="hid", bufs=2))
                stage = rep_stack.enter_context(tc.tile_pool(name="stage", bufs=5))
                ag_pool = rep_stack.enter_context(tc.tile_pool(name="ag", bufs=4))
                ostg = rep_stack.enter_context(tc.tile_pool(name="ostg", bufs=3))

                hid_src = hiddenT.rearrange("(c p) t -> p c t", p=P)
                hid_tiles = {}

                def load_hid(e):
                    if e >= 8 or e in hid_tiles:
                        return
                    ta, tb = e * TE, (e + 1) * TE
                    hid_e = hid_pool.tile([P, KC * TE], BF16, tag="hidc",
                                          name=f"hid{rep}_{e}")
                    hid_dst = hid_e.rearrange("p (c t) -> p c t", t=TE)
                    for c0 in range(0, KC, 8):
                        nc.sync.dma_start(hid_dst[:, c0:c0 + 8, :],
                                          hid_src[:, c0:c0 + 8, ta:tb])
                    hid_tiles[e] = hid_e

                def emit_qkv_eighth(e):
                    ta, tb = e * TE, (e + 1) * TE
                    load_hid(e)
                    hid_e = hid_tiles.pop(e)
                    load_hid(e + 1)          # prefetch next eighth
                    for pair, mbs in enumerate([(4, 5), (0, 1), (2, 3)]):
                        psums = [gemm_pool.tile([P, TE], F32, tag="gemm",
                                                name=f"pj{rep}_{e}_{pair}_{u}")
                                 for u in range(2)]
                        for c in range(KC):
                            for u in range(2):
                                mb = mbs[u]
                                lhsT = wqkv_sb[:, c * WQ + mb * P:c * WQ + (mb + 1) * P]
                                nc.tensor.matmul(
                                    psums[u][:, :], lhsT=lhsT,
                                    rhs=hid_e[:, c * TE:(c + 1) * TE],
                                    start=(c == 0), stop=(c == KC - 1))
                        for u in range(2):
                            mb = mbs[u]
                            if mb < 5:
                                # rope: qk[d] = raw[d]*cos2[d] + raw[(d+64)%128]*sin2[d]
                                raw = stage.tile([P, TE], F32, tag="stg", name="raw")
                                nc.vector.tensor_copy(raw[:], psums[u][:])
                                rot = stage.tile([P, TE], F32, tag="stg", name="rot")
                                nc.gpsimd.dma_start(rot[0:64, :], raw[64:128, :])
                                nc.gpsimd.dma_start(rot[64:128, :], raw[0:64, :])
                                t1 = stage.tile([P, TE], F32, tag="stg", name="t1")
                                nc.vector.tensor_mul(t1[:], raw[:], cos2_sb[:, ta:tb])
                                t2 = stage.tile([P, TE], F32, tag="stg", name="t2")
                                nc.vector.tensor_mul(t2[:], rot[:], sin2_sb[:, ta:tb])
                                nc.vector.tensor_add(qk_sb[mb][:, ta:tb], t1[:], t2[:])
                            else:
                                # v: evacuate bf16 [d, t], transpose each [d, tk]
                                # chunk into v_sb's [tk, d] slot (ones col kept)
                                vstg = stage.tile([P, TE], BF16, tag="vstg",
                                                  name="vstg")
                                nc.vector.tensor_copy(vstg[:], psums[u][:])
                                for ct in range(TE // P):
                                    gc = e * (TE // P) + ct
                                    nc.sync.dma_start_transpose(
                                        v_sb[:, gc * VW:gc * VW + P],
                                        vstg[:, ct * P:(ct + 1) * P])

                kT = qk_sb[4]

                def emit_attn_head(b, h):
                    """Attention for tq block b (tq0=512b), local q head h."""
                    tq0 = b * TQ
                    hb = b // 2
                    qT = qk_sb[h]
                    ncv = (tq0 + TQ) // P     # contributing tk chunks
                    # ---- pass A: scores + exp -> P strips ----
                    strips = []
                    for c in range(ncv):
                        off = max(tq0, P * c)
                        w = tq0 + TQ - off
                        st = st_pool.tile([P, TQ], F32, tag="st",
                                          name=f"st{rep}_{b}_{h}_{c}")
                        nc.tensor.matmul(st[:, 0:w],
                                         lhsT=kT[:, c * P:(c + 1) * P],
                                         rhs=qT[:, off:off + w],
                                         start=True, stop=True)
                        if P * c >= tq0:
                            # strip starts on the diagonal: mask tq < tk
                            nc.vector.tensor_add(st[:, 0:P], st[:, 0:P], mask_sb[:])
                        pt = p_pool.tile([P, TQ], BF16, tag="p",
                                         name=f"p{rep}_{b}_{h}_{c}")
                        nc.scalar.activation(pt[:, 0:w], st[:, 0:w],
                                             mybir.ActivationFunctionType.Exp,
                                             scale=SCALE)
                        if phases == 4 and h == 0 and b == 0 and c < 2:
                            dp = misc.tile([P, TQ], F32, tag="dbgp", name="dp")
                            nc.vector.tensor_copy(dp[:, 0:w], pt[:, 0:w])
                            nc.scalar.dma_start(
                                outp[(2 + c) * P:(3 + c) * P, 0:w], dp[:, 0:w])
                        strips.append((pt, off, w))
                    # ---- pass B: stationary-P PV, row sums free in col 128 ----
                    tstg = ts_pool.tile([P, TQ], BF16, tag="ts", name="tstg")
                    for j in range(NJ):
                        jg = tq0 // P + j     # global tq chunk
                        pv = pv_pool.tile([P, 512], F32, tag="pv",
                                          name=f"pv{rep}_{b}_{h}_{j}")
                        for c in range(jg + 1):
                            pt, off, w = strips[c]
                            a = jg * P - off
                            nc.tensor.matmul(
                                pv[:, 0:SW],
                                lhsT=pt[:, a:a + P],
                                rhs=v_sb[:, c * VW:c * VW + SW],
                                start=(c == 0), stop=(c == jg))
                        # normalize rows by col-P sums; result is [tq, d]
                        inv = misc.tile([P, 1], F32, tag="inv", name="inv")
                        nc.vector.reciprocal(inv[:], pv[:, P:P + 1])
                        normed = misc.tile([P, P], BF16, tag="nrm", name="normed")
                        nc.vector.tensor_scalar_mul(normed[:], pv[:, 0:P], inv[:])
                        if phases == 4 and h == 0:
                            # debug: normed chunk (no transpose) + l column
                            dn = misc.tile([P, P], F32, tag="dbgn", name="dn")
                            nc.vector.tensor_copy(dn[:], normed[:])
                            nc.scalar.dma_start(outp[0:P, jg * P:(jg + 1) * P],
                                                dn[:])
                            dl = misc.tile([P, 1], F32, tag="dbgl", name="dl")
                            nc.vector.tensor_copy(dl[:], pv[:, P:P + 1])
                            nc.scalar.dma_start(outp[P:2 * P, jg * P:jg * P + 1],
                                                dl[:])
                        # flip back to [d, tq]
                        nc.sync.dma_start_transpose(tstg[:, j * P:(j + 1) * P],
                                                    normed[:])
                    nc.scalar.dma_start(
                        attn_bounce[hb][h * P:(h + 1) * P,
                                        tq0 - hb * TH:tq0 - hb * TH + TQ],
                        tstg[:])
                    if phases == 2:
                        # debug: dump per-head attention output to outp (as f32)
                        dbg = misc.tile([P, TQ], F32, tag="dbgf", name="dbgf")
                        nc.vector.tensor_copy(dbg[:], tstg[:])
                        nc.scalar.dma_start(outp[h * P:(h + 1) * P, tq0:tq0 + TQ],
                                            dbg[:])

                def emit_gather(hb):
                    nc.gpsimd.collective_compute(
                        "AllGather",
                        mybir.AluOpType.bypass,
                        ins=[attn_bounce[hb][:]],
                        outs=[gathered[hb][:]],
                        replica_groups=[list(range(N_CORES))],
                    )

                def emit_oproj_tile(tq):
                    ghalf = (tq * QT) // TH
                    qa = tq * QT - ghalf * TH      # offset within gathered
                    psums = [gemm_pool.tile([P, QT], F32, tag="gemm",
                                            name=f"po{rep}_{tq}_{mb}")
                             for mb in range(QH)]
                    ag_src = gathered[ghalf].rearrange("(c p) t -> p c t", p=P)
                    for cg in range(0, JC, 4):
                        ag_t = ag_pool.tile([P, 4 * QT], BF16, tag="ag",
                                            name="ag_t")
                        ag_dst = ag_t.rearrange("p (c t) -> p c t", t=QT)
                        eng = nc.scalar if (cg // 4) % 2 else nc.sync
                        eng.dma_start(ag_dst[:, :, :],
                                      ag_src[:, cg:cg + 4, qa:qa + QT])
                        for ci in range(4):
                            c = cg + ci
                            for mb in range(QH):
                                lhsT = wo_sb[:, c * WO + mb * P:c * WO + (mb + 1) * P]
                                nc.tensor.matmul(
                                    psums[mb][:, :], lhsT=lhsT,
                                    rhs=ag_t[:, ci * QT:(ci + 1) * QT],
                                    start=(c == 0), stop=(c == JC - 1))
                    for mb in range(QH):
                        ob = ostg.tile([P, QT], F32, tag="ob", name="ob")
                        nc.vector.tensor_copy(ob[:], psums[mb][:])
                        nc.scalar.dma_start(
                            outp[mb * P:(mb + 1) * P, tq * QT:(tq + 1) * QT],
                            ob[:])

                # ================= pipelined emission =================
                for e in range(4):
                    emit_qkv_eighth(e)
                full = phases == 3
                if phases >= 2:
                    for h in range(QH):
                        emit_attn_head(0, h)
                emit_qkv_eighth(4)
                emit_qkv_eighth(5)
                if phases >= 2:
                    for h in range(QH):
                        emit_attn_head(1, h)
                    if full:
                        emit_gather(0)
                emit_qkv_eighth(6)
                emit_qkv_eighth(7)
                if phases == 1:
                    # debug: dump rope'd q heads + k to outp (rows 0..511 = q0..q3)
                    dbg_pool = rep_stack.enter_context(
                        tc.tile_pool(name="dbg", bufs=1))
                    for h in range(4):
                        dtile = dbg_pool.tile([P, t_len], F32, tag=f"dbg{h}")
                        nc.vector.tensor_copy(dtile[:], qk_sb[h][:])
                        nc.scalar.dma_start(outp[h * P:(h + 1) * P, :], dtile[:])
                if phases >= 2:
                    for h in range(QH):
                        emit_attn_head(2, h)
                    emit_attn_head(3, 0)
                    emit_attn_head(3, 1)
                    if full:
                        emit_oproj_tile(0)
                    emit_attn_head(3, 2)
                    emit_attn_head(3, 3)
                    if full:
                        emit_gather(1)
                        for tq in range(1, t_len // QT):
                            emit_oproj_tile(tq)

    nc.compile()
    return nc


def make_inputs(positions, hidden_states, w_qkv, w_o):
    """Host-side shard + relayout.  Returns per-core input maps."""
    half = D // 2
    inv_freq = 1.0 / (1e6 ** (np.arange(0, half, dtype=np.float32) / half))
    freqs = positions.astype(np.float32)[:, None] * inv_freq[None, :]
    cosT = np.cos(freqs).T.astype(np.float32)      # [64, T]
    sinT = np.sin(freqs).T.astype(np.float32)
    cos2 = np.ascontiguousarray(np.concatenate([cosT, cosT], axis=0))
    sin2 = np.ascontiguousarray(np.concatenate([-sinT, sinT], axis=0))

    ii = np.arange(P)
    maskd = np.where(ii[None, :] >= ii[:, None], 0.0, NEG).astype(np.float32)

    hiddenT = np.ascontiguousarray(hidden_states.T).astype(bf16)

    q_size = 32 * D
    in_maps = []
    for i in range(N_CORES):
        rows = np.concatenate([
            w_qkv[QH * P * i:QH * P * (i + 1)],                      # 4 q heads
            w_qkv[q_size + P * i:q_size + P * (i + 1)],              # k head
            w_qkv[q_size + 8 * D + P * i:q_size + 8 * D + P * (i + 1)],  # v head
        ], axis=0)
        wqkvT_i = np.ascontiguousarray(rows.T).astype(bf16)
        woT_i = np.ascontiguousarray(w_o[QH * P * i:QH * P * (i + 1), :].T).astype(bf16)
        in_maps.append({
            "hiddenT": hiddenT,
            "wqkvT": wqkvT_i,
            "woT": woT_i,
            "cos2": cos2,
            "sin2": sin2,
            "maskd": maskd,
        })
    return in_maps


def assemble(results, t_len=2048):
    final = np.empty((t_len, N_CORES * QH * P), dtype=np.float32)
    for i in range(N_CORES):
        final[:, QH * P * i:QH * P * (i + 1)] = results[i]["outp"].T
    return final


def kernel(positions, hidden_states, w_qkv, w_o):
    positions = np.asarray(positions)
    hidden_states = np.asarray(hidden_states, dtype=np.float32)
    w_qkv = np.asarray(w_qkv, dtype=np.float32)
    w_o = np.asarray(w_o, dtype=np.float32)
    t_len = hidden_states.shape[0]

    nc = build_nc(t_len)
    in_maps = make_inputs(positions, hidden_states, w_qkv, w_o)
    res = run_bass_kernel_spmd(nc, in_maps, list(range(N_CORES)))
    return assemble(res.results, t_len)


# revision 5
# speedup vs baseline: 2.2294x; 1.6125x over previous
"""Mixtral-style GQA attention block, tensor-parallel over 8 NeuronCores.

Sharding: core i owns q heads 4i..4i+3 and kv head i (GQA group == 4, so the
kv head's whole group lives on one core).  w_qkv is column-sharded by head,
w_o is row-sharded; the only collective is an AllGather of the per-core
attention outputs (bf16, 2MB per core).  Each core then computes a disjoint
512-column slice of the final output, so the host-side unshard is a pure
concatenation.

v2 layout: the whole rep is software-pipelined in emission order —
QKV runs in t-quarters with attention tq-blocks, the two AllGathers and
o_proj tiles interleaved between them so softmax exp (Act engine) and the
collectives hide under GEMM streaming.  PV streams V||ones against a
stationary probability tile, so softmax row-sums come free in psum column
128 (no separate ones-matmul, no broadcast matmul); normalization is a
per-partition scalar multiply and the [tq, d] result is flipped back to
[d, tq] with xbar DMA transposes.

All matmuls run in bf16 (fp32 PSUM accumulation); softmax runs in fp32
without max-subtraction (scores are ~N(0,1) by construction, exp cannot
overflow).
"""

import numpy as np
import ml_dtypes
from contextlib import ExitStack

import concourse.bass as bass
import concourse.mybir as mybir
import concourse.tile as tile
from concourse import bacc
from concourse.bass_utils import run_bass_kernel_spmd

P = 128
HID = 4096
D = 128
QH = 4                      # local q heads per core
NB = 6                      # projection M-blocks: q0..q3, k, v
KC = HID // P               # contraction chunks over hidden dim
N_CORES = 8
SCALE = float(D) ** -0.5
NEG = -1.0e30

dt = mybir.dt
bf16 = ml_dtypes.bfloat16

F32 = dt.float32
BF16 = dt.bfloat16


def build_nc(t_len=2048, phases=3, reps=1):
    TCH = t_len // P            # token chunks (16)
    TE = t_len // 8             # eighth width (256) — qkv chunk granularity
    TQ = t_len // 4             # attention block width (512)
    TH = t_len // 2             # half width (1024) — allgather granularity
    QT = 256                    # o_proj t-tile width
    NJ = TQ // P                # tq chunks per attention block (4)
    WQ = NB * P                 # 768
    WO = QH * P                 # 512
    VW = 2 * P                  # v chunk stride: xbar-transpose dests need 256B align
    SW = P + 8                  # streamed width per v chunk: v(128) + ones + 7 pad

    nc = bacc.Bacc("TRN2", target_bir_lowering=False, debug=False,
                   num_devices=N_CORES)

    hiddenT = nc.dram_tensor("hiddenT", [HID, t_len], BF16, kind="ExternalInput").ap()
    wqkvT = nc.dram_tensor("wqkvT", [HID, WQ], BF16, kind="ExternalInput").ap()
    woT = nc.dram_tensor("woT", [HID, WO], BF16, kind="ExternalInput").ap()
    cos2 = nc.dram_tensor("cos2", [P, t_len], F32, kind="ExternalInput").ap()
    sin2 = nc.dram_tensor("sin2", [P, t_len], F32, kind="ExternalInput").ap()
    maskd = nc.dram_tensor("maskd", [P, P], F32, kind="ExternalInput").ap()
    outp = nc.dram_tensor("outp", [WO, t_len], F32, kind="ExternalOutput").ap()

    with tile.TileContext(nc) as tc:
        with ExitStack() as whole:
            persist = whole.enter_context(tc.tile_pool(name="persist", bufs=1))
            dram = whole.enter_context(tc.tile_pool(name="dram", bufs=1, space="DRAM"))

            # ---- constants ----
            cos2_sb = persist.tile([P, t_len], F32, tag="cos2")
            sin2_sb = persist.tile([P, t_len], F32, tag="sin2")
            mask_sb = persist.tile([P, P], F32, tag="mask")
            nc.sync.dma_start(cos2_sb[:], cos2[:])
            nc.sync.dma_start(sin2_sb[:], sin2[:])
            nc.sync.dma_start(mask_sb[:], maskd[:])

            # ---- persistent activations ----
            qk_sb = [persist.tile([P, t_len], BF16, tag=f"qk{mb}", name=f"qk{mb}")
                     for mb in range(5)]
            # v chunks interleaved with a ones column: chunk c at
            # [:, c*VW : c*VW+P], ones at [:, c*VW+P]
            v_sb = persist.tile([P, TCH * VW], BF16, tag="v", name="v_sb")
            for gc in range(TCH):
                nc.vector.memset(v_sb[:, gc * VW + P:gc * VW + P + 1], 1.0)
                nc.vector.memset(v_sb[:, gc * VW + P + 1:gc * VW + SW], 0.0)

            # ---- persistent weight caches (shared by all reps) ----
            JC = N_CORES * QH
            wqkv_sb = persist.tile([P, KC * WQ], BF16, tag="wqkv")
            wqkv_src = wqkvT.rearrange("(c p) w -> p c w", p=P)
            wqkv_dst = wqkv_sb.rearrange("p (c w) -> p c w", w=WQ)
            for c0 in range(0, KC, 8):
                nc.scalar.dma_start(wqkv_dst[:, c0:c0 + 8, :],
                                    wqkv_src[:, c0:c0 + 8, :])
            wo_sb = persist.tile([P, JC * WO], BF16, tag="wo")
            wo_src = woT.rearrange("(c p) w -> p c w", p=P)
            wo_dst = wo_sb.rearrange("p (c w) -> p c w", w=WO)
            for c0 in range(0, JC, 8):
                nc.scalar.dma_start(wo_dst[:, c0:c0 + 8, :],
                                    wo_src[:, c0:c0 + 8, :])

            for rep in range(reps):
              with ExitStack() as rep_stack:
                attn_bounce = [dram.tile([QH * P, TQ], BF16,
                                         tag=f"attn_bounce{rep}_{b}",
                                         name=f"attn_bounce{rep}_{b}")
                               for b in range(4)]
                gathered = [dram.tile([N_CORES * QH * P, TQ], BF16,
                                      tag=f"gathered{rep}_{b}",
                                      name=f"gathered{rep}_{b}",
                                      addr_space="Shared")
                            for b in range(4)]

                # ---- all pools allocated at rep start in fixed order, so
                # each rep's pools land on the same regions and WAR only
                # against their own previous-rep usage (QKV psum frees early,
                # o_proj psum frees last — no cross-pool collisions)
                gemm_pool = rep_stack.enter_context(
                    tc.tile_pool(name="gemm_psum", bufs=4, space="PSUM"))
                st_pool = rep_stack.enter_context(
                    tc.tile_pool(name="st_psum", bufs=2, space="PSUM"))
                pv_pool = rep_stack.enter_context(
                    tc.tile_pool(name="pv_psum", bufs=2, space="PSUM"))
                p_pool = rep_stack.enter_context(tc.tile_pool(name="pstrips", bufs=20))
                misc = rep_stack.enter_context(tc.tile_pool(name="attn_misc", bufs=4))
                ts_pool = rep_stack.enter_context(tc.tile_pool(name="tstage", bufs=2))
                hid_pool = rep_stack.enter_context(tc.tile_pool(name="hid", bufs=2))
                stage = rep_stack.enter_context(tc.tile_pool(name="stage", bufs=5))
                ag_pool = rep_stack.enter_context(tc.tile_pool(name="ag", bufs=4))
                ostg = rep_stack.enter_context(tc.tile_pool(name="ostg", bufs=3))

                hid_src = hiddenT.rearrange("(c p) t -> p c t", p=P)
                hid_tiles = {}

                def load_hid(e):
                    if e >= 8 or e in hid_tiles:
                        return
                    ta, tb = e * TE, (e + 1) * TE
                    hid_e = hid_pool.tile([P, KC * TE], BF16, tag="hidc",
                                          name=f"hid{rep}_{e}")
                    hid_dst = hid_e.rearrange("p (c t) -> p c t", t=TE)
                    for c0 in range(0, KC, 8):
                        nc.sync.dma_start(hid_dst[:, c0:c0 + 8, :],
                                          hid_src[:, c0:c0 + 8, ta:tb])
                    hid_tiles[e] = hid_e

                def emit_qkv_eighth(e):
                    ta, tb = e * TE, (e + 1) * TE
                    load_hid(e)
                    hid_e = hid_tiles.pop(e)
                    load_hid(e + 1)          # prefetch next eighth
                    for pair, mbs in enumerate([(4, 5), (0, 1), (2, 3)]):
                        psums = [gemm_pool.tile([P, TE], F32, tag="gemm",
                                                name=f"pj{rep}_{e}_{pair}_{u}")
                                 for u in range(2)]
                        for c in range(KC):
                            for u in range(2):
                                mb = mbs[u]
                                lhsT = wqkv_sb[:, c * WQ + mb * P:c * WQ + (mb + 1) * P]
                                nc.tensor.matmul(
                                    psums[u][:, :], lhsT=lhsT,
                                    rhs=hid_e[:, c * TE:(c + 1) * TE],
                                    start=(c == 0), stop=(c == KC - 1))
                        for u in range(2):
                            mb = mbs[u]
                            if mb < 5:
                                # rope: qk[d] = raw[d]*cos2[d] + raw[(d+64)%128]*sin2[d]
                                raw = stage.tile([P, TE], F32, tag="stg", name="raw")
                                nc.vector.tensor_copy(raw[:], psums[u][:])
                                rot = stage.tile([P, TE], F32, tag="stg", name="rot")
                                nc.gpsimd.dma_start(rot[0:64, :], raw[64:128, :])
                                nc.gpsimd.dma_start(rot[64:128, :], raw[0:64, :])
                                t1 = stage.tile([P, TE], F32, tag="stg", name="t1")
                                nc.vector.tensor_mul(t1[:], raw[:], cos2_sb[:, ta:tb])
                                t2 = stage.tile([P, TE], F32, tag="stg", name="t2")
                                nc.vector.tensor_mul(t2[:], rot[:], sin2_sb[:, ta:tb])
                                nc.vector.tensor_add(qk_sb[mb][:, ta:tb], t1[:], t2[:])
                            else:
                                # v: evacuate bf16 [d, t], transpose each [d, tk]
                                # chunk into v_sb's [tk, d] slot (ones col kept)
                                vstg = stage.tile([P, TE], BF16, tag="vstg",
                                                  name="vstg")
                                nc.vector.tensor_copy(vstg[:], psums[u][:])
                                for ct in range(TE // P):
                                    gc = e * (TE // P) + ct
                                    nc.sync.dma_start_transpose(
                                        v_sb[:, gc * VW:gc * VW + P],
                                        vstg[:, ct * P:(ct + 1) * P])

                kT = qk_sb[4]

                def emit_attn_head(b, h):
                    """Attention for tq block b (tq0=512b), local q head h."""
                    tq0 = b * TQ
                    hb = b // 2
                    qT = qk_sb[h]
                    ncv = (tq0 + TQ) // P     # contributing tk chunks
                    # ---- pass A: scores + exp -> P strips ----
                    strips = []
                    for c in range(ncv):
                        off = max(tq0, P * c)
                        w = tq0 + TQ - off
                        st = st_pool.tile([P, TQ], F32, tag="st",
                                          name=f"st{rep}_{b}_{h}_{c}")
                        nc.tensor.matmul(st[:, 0:w],
                                         lhsT=kT[:, c * P:(c + 1) * P],
                                         rhs=qT[:, off:off + w],
                                         start=True, stop=True)
                        if P * c >= tq0:
                            # strip starts on the diagonal: mask tq < tk
                            nc.vector.tensor_add(st[:, 0:P], st[:, 0:P], mask_sb[:])
                        pt = p_pool.tile([P, TQ], BF16, tag="p",
                                         name=f"p{rep}_{b}_{h}_{c}")
                        nc.scalar.activation(pt[:, 0:w], st[:, 0:w],
                                             mybir.ActivationFunctionType.Exp,
                                             scale=SCALE)
                        if phases == 4 and h == 0 and b == 0 and c < 2:
                            dp = misc.tile([P, TQ], F32, tag="dbgp", name="dp")
                            nc.vector.tensor_copy(dp[:, 0:w], pt[:, 0:w])
                            nc.scalar.dma_start(
                                outp[(2 + c) * P:(3 + c) * P, 0:w], dp[:, 0:w])
                        strips.append((pt, off, w))
                    # ---- pass B: stationary-P PV, row sums free in col 128 ----
                    tstg = ts_pool.tile([P, TQ], BF16, tag="ts", name="tstg")
                    for j in range(NJ):
                        jg = tq0 // P + j     # global tq chunk
                        pv = pv_pool.tile([P, 512], F32, tag="pv",
                                          name=f"pv{rep}_{b}_{h}_{j}")
                        for c in range(jg + 1):
                            pt, off, w = strips[c]
                            a = jg * P - off
                            nc.tensor.matmul(
                                pv[:, 0:SW],
                                lhsT=pt[:, a:a + P],
                                rhs=v_sb[:, c * VW:c * VW + SW],
                                start=(c == 0), stop=(c == jg))
                        # normalize rows by col-P sums; result is [tq, d]
                        inv = misc.tile([P, 1], F32, tag="inv", name="inv")
                        nc.vector.reciprocal(inv[:], pv[:, P:P + 1])
                        normed = misc.tile([P, P], BF16, tag="nrm", name="normed")
                        nc.vector.tensor_scalar_mul(normed[:], pv[:, 0:P], inv[:])
                        if phases == 4 and h == 0:
                            # debug: normed chunk (no transpose) + l column
                            dn = misc.tile([P, P], F32, tag="dbgn", name="dn")
                            nc.vector.tensor_copy(dn[:], normed[:])
                            nc.scalar.dma_start(outp[0:P, jg * P:(jg + 1) * P],
                                                dn[:])
                            dl = misc.tile([P, 1], F32, tag="dbgl", name="dl")
                            nc.vector.tensor_copy(dl[:], pv[:, P:P + 1])
                            nc.scalar.dma_start(outp[P:2 * P, jg * P:jg * P + 1],
                                                dl[:])
                        # flip back to [d, tq]
                        nc.sync.dma_start_transpose(tstg[:, j * P:(j + 1) * P],
                                                    normed[:])
                    nc.scalar.dma_start(
                        attn_bounce[b][h * P:(h + 1) * P, 0:TQ], tstg[:])
                    if phases == 2:
                        # debug: dump per-head attention output to outp (as f32)
                        dbg = misc.tile([P, TQ], F32, tag="dbgf", name="dbgf")
                        nc.vector.tensor_copy(dbg[:], tstg[:])
                        nc.scalar.dma_start(outp[h * P:(h + 1) * P, tq0:tq0 + TQ],
                                            dbg[:])

                def emit_gather(b):
                    nc.gpsimd.collective_compute(
                        "AllGather",
                        mybir.AluOpType.bypass,
                        ins=[attn_bounce[b][:]],
                        outs=[gathered[b][:]],
                        replica_groups=[list(range(N_CORES))],
                    )

                def emit_oproj_tile(tq):
                    ghalf = (tq * QT) // TQ        # source quarter-gather
                    qa = tq * QT - ghalf * TQ      # offset within gathered
                    psums = [gemm_pool.tile([P, QT], F32, tag="gemm",
                                            name=f"po{rep}_{tq}_{mb}")
                             for mb in range(QH)]
                    ag_src = gathered[ghalf].rearrange("(c p) t -> p c t", p=P)
                    for cg in range(0, JC, 4):
                        ag_t = ag_pool.tile([P, 4 * QT], BF16, tag="ag",
                                            name="ag_t")
                        ag_dst = ag_t.rearrange("p (c t) -> p c t", t=QT)
                        eng = nc.scalar if (cg // 4) % 2 else nc.sync
                        eng.dma_start(ag_dst[:, :, :],
                                      ag_src[:, cg:cg + 4, qa:qa + QT])
                        for ci in range(4):
                            c = cg + ci
                            for mb in range(QH):
                                lhsT = wo_sb[:, c * WO + mb * P:c * WO + (mb + 1) * P]
                                nc.tensor.matmul(
                                    psums[mb][:, :], lhsT=lhsT,
                                    rhs=ag_t[:, ci * QT:(ci + 1) * QT],
                                    start=(c == 0), stop=(c == JC - 1))
                    for mb in range(QH):
                        ob = ostg.tile([P, QT], F32, tag="ob", name="ob")
                        nc.vector.tensor_copy(ob[:], psums[mb][:])
                        nc.scalar.dma_start(
                            outp[mb * P:(mb + 1) * P, tq * QT:(tq + 1) * QT],
                            ob[:])

                # ================= pipelined emission =================
                for e in range(4):
                    emit_qkv_eighth(e)
                full = phases == 3
                if phases >= 2:
                    for h in range(QH):
                        emit_attn_head(0, h)
                    if full:
                        emit_gather(0)
                emit_qkv_eighth(4)
                emit_qkv_eighth(5)
                if phases >= 2:
                    for h in range(QH):
                        emit_attn_head(1, h)
                    if full:
                        emit_gather(1)
                emit_qkv_eighth(6)
                emit_qkv_eighth(7)
                if phases == 1:
                    # debug: dump rope'd q heads + k to outp (rows 0..511 = q0..q3)
                    dbg_pool = rep_stack.enter_context(
                        tc.tile_pool(name="dbg", bufs=1))
                    for h in range(4):
                        dtile = dbg_pool.tile([P, t_len], F32, tag=f"dbg{h}")
                        nc.vector.tensor_copy(dtile[:], qk_sb[h][:])
                        nc.scalar.dma_start(outp[h * P:(h + 1) * P, :], dtile[:])
                if phases >= 2:
                    for h in range(QH):
                        emit_attn_head(2, h)
                    if full:
                        emit_gather(2)
                    emit_attn_head(3, 0)
                    emit_attn_head(3, 1)
                    if full:
                        emit_oproj_tile(0)
                    emit_attn_head(3, 2)
                    emit_attn_head(3, 3)
                    if full:
                        emit_gather(3)
                        for tq in range(1, t_len // QT):
                            emit_oproj_tile(tq)

    nc.compile()
    return nc


def make_inputs(positions, hidden_states, w_qkv, w_o):
    """Host-side shard + relayout.  Returns per-core input maps."""
    half = D // 2
    inv_freq = 1.0 / (1e6 ** (np.arange(0, half, dtype=np.float32) / half))
    freqs = positions.astype(np.float32)[:, None] * inv_freq[None, :]
    cosT = np.cos(freqs).T.astype(np.float32)      # [64, T]
    sinT = np.sin(freqs).T.astype(np.float32)
    cos2 = np.ascontiguousarray(np.concatenate([cosT, cosT], axis=0))
    sin2 = np.ascontiguousarray(np.concatenate([-sinT, sinT], axis=0))

    ii = np.arange(P)
    maskd = np.where(ii[None, :] >= ii[:, None], 0.0, NEG).astype(np.float32)

    hiddenT = np.ascontiguousarray(hidden_states.T).astype(bf16)

    q_size = 32 * D
    in_maps = []
    for i in range(N_CORES):
        rows = np.concatenate([
            w_qkv[QH * P * i:QH * P * (i + 1)],                      # 4 q heads
            w_qkv[q_size + P * i:q_size + P * (i + 1)],              # k head
            w_qkv[q_size + 8 * D + P * i:q_size + 8 * D + P * (i + 1)],  # v head
        ], axis=0)
        wqkvT_i = np.ascontiguousarray(rows.T).astype(bf16)
        woT_i = np.ascontiguousarray(w_o[QH * P * i:QH * P * (i + 1), :].T).astype(bf16)
        in_maps.append({
            "hiddenT": hiddenT,
            "wqkvT": wqkvT_i,
            "woT": woT_i,
            "cos2": cos2,
            "sin2": sin2,
            "maskd": maskd,
        })
    return in_maps


def assemble(results, t_len=2048):
    final = np.empty((t_len, N_CORES * QH * P), dtype=np.float32)
    for i in range(N_CORES):
        final[:, QH * P * i:QH * P * (i + 1)] = results[i]["outp"].T
    return final


def kernel(positions, hidden_states, w_qkv, w_o):
    positions = np.asarray(positions)
    hidden_states = np.asarray(hidden_states, dtype=np.float32)
    w_qkv = np.asarray(w_qkv, dtype=np.float32)
    w_o = np.asarray(w_o, dtype=np.float32)
    t_len = hidden_states.shape[0]

    nc = build_nc(t_len)
    in_maps = make_inputs(positions, hidden_states, w_qkv, w_o)
    res = run_bass_kernel_spmd(nc, in_maps, list(range(N_CORES)))
    return assemble(res.results, t_len)
